# revision 19
# baseline (speedup 1.0000x reference)
"""Causal MHA (batch=4, seq=2048, dim=1024, 16 heads x 64) on 8 TRN2 NeuronCores.

Sharding: core c handles batch b = c//2 and head-group g = c%2 (8 heads).
Each core computes QKV projections for its heads, causal attention, and a
partial output projection over its 512 features. The host sums the two
partial projections per batch and transposes back.

All matmuls run in bf16 (fp32 PSUM accumulate); softmax runs without max
subtraction (logits are bounded ~|8|), with the row sums produced by an
extra ones-column appended to V during the PV matmul.

Schedule: per (head-pair, q-chunk) the softmax-normalization chain
(ln/exp recip of the PV sums row, a rank-2 selector matmul that
broadcasts both halves' recip rows across 128 partitions, and the DVE
normalize-multiplies) is pipelined one pair behind, emitted between the
next pair's off-diagonal sims and its diagonal sims, so neither PE nor
the DVE mask-mul FIFO ever waits on it. Diagonal tiles stream only
their valid q columns through the PE on both the sim and PV matmuls.
"""
import sys

sys.path.insert(0, "/opt/trn_rl_repo")

import json
import numpy as np
import ml_dtypes
from contextlib import ExitStack

import concourse.bass as bass
import concourse.tile as tile
from concourse import mybir
from concourse import bass_utils as _bu
from concourse.bass_utils import run_bass_kernel_spmd

LDW_OPT = False  # walrus ldw-opt rejects bass-emitted Ldweights outright

BF16 = mybir.dt.bfloat16
F32 = mybir.dt.float32
F32R = mybir.dt.float32r
Exp = mybir.ActivationFunctionType.Exp
Ln = mybir.ActivationFunctionType.Ln

DIM = 1024
SEQ = 2048
NH = 16          # total heads
HPC = 8          # heads per core
DH = 64          # head dim
SCALE = DH ** -0.5
NCORES = 8
FPC = HPC * DH   # features per core = 512
NKT = SEQ // 128   # 16 k-tiles of 128
NQC = SEQ // 512   # 4 q-chunks of 512
VSTRIDE = DH + 2   # 66: V columns per head incl. ones col + pad

_WALRUS_PATCHED = False


def _patch_walrus_wait_limit():
    """This container's walrus rejects >1 sem wait per instruction
    (CoreV3 setupSyncWait). Tile's tail drain carries one wait per live
    proc; split the extras into preceding single-wait Drain carriers at
    BIR-JSON serialization time."""
    global _WALRUS_PATCHED
    if _WALRUS_PATCHED:
        return
    _WALRUS_PATCHED = True

    if LDW_OPT:
        orig_run = _bu.run_command

        def run_patched(cmd, *a, **k):
            cmd = ["--enable-ldw-opt=true" if c == "--enable-ldw-opt=false" else c
                   for c in cmd]
            return orig_run(cmd, *a, **k)

        _bu.run_command = run_patched

    orig = bass.Bass.to_json_bytes

    def _merge_ldw_halves(insts):
        """Fold row-tiled Ldweights pairs ([64,128] at row 0 + [64,128] at
        row 64 of the same tensor) into one [128,128] load carrying both
        halves' waits."""
        out = []
        pend = None  # (index_in_out, inst) of a candidate row-0 half
        for inst in insts:
            op = inst["opcode"]
            if inst.get("engine") != "PE":
                out.append(inst)
                continue
            if op == "Ldweights" and inst.get("tile_size") == [64, 128]:
                ap = inst["ins"][0].get("ap")
                if inst.get("tile_position") == [0, 0] and ap and ap[0][1] == 64:
                    out.append(inst)
                    pend = (len(out) - 1, inst)
                    continue
                if (pend is not None
                        and inst.get("tile_position") == [64, 0] and ap
                        and ap[0][1] == 64):
                    a = pend[1]
                    aap = a["ins"][0]["ap"]
                    same = (a["ins"][0].get("memref") == inst["ins"][0].get("memref")
                            and aap[0][0] == ap[0][0] and aap[1] == ap[1]
                            and inst["ins"][0].get("offset", 0)
                            == a["ins"][0].get("offset", 0) + 64 * aap[0][0])
                    b_si = inst.get("sync_info") or {}
                    if same and not b_si.get("on_update"):
                        aap[0][1] = 128
                        a["tile_size"] = [128, 128]
                        a.setdefault("sync_info", {"on_update": [], "on_wait": []})
                        a["sync_info"].setdefault("on_wait", [])
                        a["sync_info"]["on_wait"].extend(b_si.get("on_wait") or [])
                        pend = None
                        continue
                out.append(inst)
                pend = None
            else:
                if op not in ("Matmult", "NoOp"):
                    pend = None
                out.append(inst)
        return out

    def patched(self, *a, **k):
        d = json.loads(orig(self, *a, **k))
        for f in d["functions"]:
            for bb in f["blocks"]:
                bb["instructions"] = _merge_ldw_halves(bb["instructions"])
                out = []
                last_ldw = None  # (key, still_valid)
                for inst in bb["instructions"]:
                    si = inst.get("sync_info")
                    ow = (si or {}).get("on_wait") or []
                    op = inst["opcode"]

                    def emit_carriers(waits):
                        for j, w in enumerate(waits):
                            out.append({
                                "name": f"{inst['name']}__w{j}",
                                "opcode": "NoOp",
                                "engine": inst["engine"],
                                "ins": [], "outs": [],
                                "debug": inst.get("debug", 0),
                                "sync_info": {"on_update": [], "on_wait": [w]},
                            })

                    # drop a Ldweights identical to the previous one when only
                    # Matmult/NoOp sit between (weights already resident);
                    # also fold the row-tiled [64,128]+[64,128] half-pair into
                    # the single [128,128] load emitted by _merge_ldw_halves
                    if op == "Ldweights" and inst["engine"] == "PE":
                        key = json.dumps(
                            [inst.get("ins"), inst.get("tile_position"),
                             inst.get("tile_size")], sort_keys=True)
                        if last_ldw == key and not (si or {}).get("on_update"):
                            emit_carriers(ow)
                            continue
                        last_ldw = key
                    elif inst["engine"] == "PE" and op not in ("Matmult", "NoOp"):
                        last_ldw = None

                    if len(ow) > 1:
                        emit_carriers(ow[:-1])
                        si["on_wait"] = [ow[-1]]
                    out.append(inst)
                bb["instructions"] = out
        return json.dumps(d).encode()

    bass.Bass.to_json_bytes = patched


def build_kernel():
    nc = bass.Bass()
    xT = nc.declare_dram_parameter("xT", [DIM, SEQ], BF16, isOutput=False)
    wq = nc.declare_dram_parameter("wq", [DIM, FPC], BF16, isOutput=False)
    wk = nc.declare_dram_parameter("wk", [DIM, FPC], BF16, isOutput=False)
    wv = nc.declare_dram_parameter("wv", [DIM, FPC], BF16, isOutput=False)
    wo = nc.declare_dram_parameter("wo", [FPC, DIM], BF16, isOutput=False)
    # causal keep masks per diagonal offset r: [r, 128, 512]
    msk = nc.declare_dram_parameter("msk", [4, 128, 512], BF16, isOutput=False)
    outT = nc.declare_dram_parameter("outT", [DIM, SEQ], BF16, isOutput=True)

    with tile.TileContext(nc) as tc, ExitStack() as ctx:
        persist = ctx.enter_context(tc.tile_pool(name="persist", bufs=1))
        work = ctx.enter_context(tc.tile_pool(name="work", bufs=4))
        pt_pool = ctx.enter_context(tc.tile_pool(name="pt", bufs=1))
        ps_mm = ctx.enter_context(tc.tile_pool(name="ps_mm", bufs=2, space="PSUM"))
        ps_s = ctx.enter_context(tc.tile_pool(name="ps_s", bufs=2, space="PSUM"))
        ps_o = ctx.enter_context(tc.tile_pool(name="ps_o", bufs=2, space="PSUM"))

        # ---- load inputs. One batched DMA per tensor (xT in two halves):
        # a [di*128+p, c] HBM row block lands at SBUF [p, di*W + c], so a
        # per-di "tile" is a column slice of one wide tile. Few DMA issues
        # (the HWDGE ring serializes issue at ~0.6us each) and the first
        # QKV chain starts as soon as wq + the first xT half land. -------
        w_wide = {}
        for name in ("wq", "wk", "wv"):
            w_wide[name] = persist.tile([128, 8 * FPC], BF16, tag=name,
                                        name=name)
        xT_wide = persist.tile([128, 8 * SEQ], BF16, tag="xT", name="xT")
        wo_wide = persist.tile([128, 4 * DIM], BF16, tag="wo", name="wo")
        msk_wide = persist.tile([128, 4 * 512], BF16, tag="msk", name="msk")

        def wsl(name, di):       # [128,FPC] view of weight block di
            return w_wide[name][:, di * FPC:(di + 1) * FPC]

        def xsl(di):             # [128,SEQ] view of xT block di
            return xT_wide[:, di * SEQ:(di + 1) * SEQ]

        # weights on the scalar HWDGE ring, xT (in 4 chunks so the QKV
        # chains can start on the first k-quarter) on the sync ring —
        # a big DMA's transfer occupies its issuing queue, so two rings
        # let weights and activations stream in parallel.
        # xT in 4 chunks on the sync HWDGE ring; weights as per-di tiles
        # (contiguous HBM rows -> full DMA bandwidth, unlike the strided
        # batched form) split across the gpsimd SWDGE and scalar HWDGE
        # queues so issue serialization doesn't delay late tensors.
        for qt in range(4):
            nc.sync.dma_start(
                xT_wide[:].rearrange("p (di c) -> p di c", di=8)[:, 2 * qt:2 * qt + 2],
                xT.ap().rearrange("(di p) c -> p di c", di=8)[:, 2 * qt:2 * qt + 2])
        for di in range(8):
            nc.gpsimd.dma_start(wsl("wq", di), wq.ap()[di * 128:(di + 1) * 128, :])
            nc.scalar.dma_start(wsl("wk", di), wk.ap()[di * 128:(di + 1) * 128, :])
        for di in range(8):
            (nc.gpsimd if di < 4 else nc.scalar).dma_start(
                wsl("wv", di), wv.ap()[di * 128:(di + 1) * 128, :])
        nc.gpsimd.dma_start(
            wo_wide[:].rearrange("p (fi c) -> p fi c", fi=4),
            wo.ap().rearrange("(fi p) c -> p fi c", fi=4))
        nc.scalar.dma_start(
            msk_wide[:].rearrange("p (r c) -> p r c", r=4),
            msk.ap().rearrange("r p c -> p r c"))
        wo_sb = [wo_wide[:, fi * DIM:(fi + 1) * DIM] for fi in range(4)]
        msk_sb = [msk_wide[:, r * 512:(r + 1) * 512] for r in range(4)]
        ones64 = persist.tile([1, DH], BF16, tag="ones64")
        nc.gpsimd.memset(ones64[:], 1.0)

        # ---- stage B: QKV projections -----------------------------------
        qk_sb = {"q": [], "k": []}
        for qn in ("q", "k"):
            for fi in range(4):
                qk_sb[qn].append(
                    persist.tile([128, SEQ], BF16, tag=f"{qn}{fi}",
                                 name=f"{qn}{fi}"))
        v_sb = [persist.tile([128, HPC * VSTRIDE], BF16, tag=f"v{ti}",
                             name=f"v{ti}") for ti in range(NKT)]

        def emit_qk(qn, wn, fi):
            # Q, K in [feature, token] layout (w stationary, xT moving).
            # Two [128,512] accumulators from the mm tag only (ps_o stays
            # free for the PV pipeline, so these chains can weave through
            # attention pairs), two token passes.
            t = qk_sb[qn][fi]
            for tp in range(2):
                ch = [ps_mm.tile([128, 512], F32, tag="mm", name=f"ch{k}")
                      for k in range(2)]
                for di in range(8):
                    for k in range(2):
                        tck = 2 * tp + k
                        nc.tensor.matmul(
                            ch[k][:], wsl(wn, di)[:, fi * 128:(fi + 1) * 128],
                            xsl(di)[:, tck * 512:(tck + 1) * 512],
                            start=(di == 0), stop=(di == 7))
                for k in range(2):
                    tck = 2 * tp + k
                    nc.vector.tensor_copy(
                        t[:, tck * 512:(tck + 1) * 512], ch[k][:])

        def emit_v(ti):
            # V in [token, feature] layout (xT stationary, wv moving), strided
            # into VSTRIDE-blocks with a ones column per head
            t = v_sb[ti]
            p = ps_mm.tile([128, 512], F32, tag="mm", name="p_v")
            for di in range(8):
                nc.tensor.matmul(
                    p[:], xsl(di)[:, ti * 128:(ti + 1) * 128],
                    wsl("wv", di),
                    start=(di == 0), stop=(di == 7))
            dst = t[:].rearrange("p (h c) -> p h c", h=HPC)[:, :, 0:DH]
            src = p[:].rearrange("p (h c) -> p h c", h=HPC)
            nc.vector.tensor_copy(dst, src)
            nc.gpsimd.memset(
                t[:].rearrange("p (h c) -> p h c", h=HPC)[:, :, DH:DH + 1], 1.0)

        ot_sb = [persist.tile([128, SEQ], BF16, tag=f"ot{fi}", name=f"ot{fi}")
                 for fi in range(4)]
        pts_map = {}

        def emit_sim(pr, ci, j0, j1):
            # S^T strips + exp into pt tiles for (head pair pr, q-chunk ci),
            # k-tiles j0..j1-1. Diagonal tiles (r >= 1) stream only their
            # valid q columns.
            q0 = ci * 512
            pts = pts_map.setdefault((pr, ci), [])
            for j in range(j0, j1):
                r = j - 4 * ci
                c0 = 128 * r if r > 0 else 0   # first valid q col in chunk
                ps = ps_s.tile([128, 1024], F32, tag="s", name="ps_st")
                for half in range(2):   # head A / head B, row-tiled
                    nc.tensor.matmul(
                        ps[:, half * 512 + c0:(half + 1) * 512],
                        qk_sb["k"][pr][half * 64:(half + 1) * 64,
                                       j * 128:(j + 1) * 128],
                        qk_sb["q"][pr][half * 64:(half + 1) * 64,
                                       q0 + c0:q0 + 512],
                        start=True, stop=True)
                pt = pt_pool.tile([128, 1024], BF16, tag=f"pt{j}", name="pt",
                                  bufs=2 if j < 8 else 1)
                pts.append(pt)
                if r < 0:
                    nc.scalar.activation(pt[:], ps[:], Exp, scale=SCALE)
                else:
                    # diagonal tile: columns ql >= 128r are valid; the
                    # rest must be zero (PV streams the full chunk on its
                    # closing matmul)
                    pt3 = pt[:].rearrange("p (b w) -> p b w", b=2)[:, :, c0:]
                    ps3 = ps[:].rearrange("p (b w) -> p b w", b=2)[:, :, c0:]
                    m3 = msk_sb[r][:, c0:][:, None, :].broadcast_to(
                        [128, 2, 512 - c0])
                    if r > 0:
                        nc.gpsimd.memset(
                            pt[:].rearrange("p (b w) -> p b w", b=2)[:, :, 0:c0],
                            0.0)
                    nc.scalar.activation(pt3, ps3, Exp, scale=SCALE)
                    nc.vector.tensor_mul(pt3, pt3, m3)

        def emit_pv(pr, ci):
            # PV: V_aug stationary [128k, 65], P^T moving.
            # Output O^T_aug [65, 512q]: rows 0:64 = O^T, row 64 = sums.
            # Diagonal tiles r in {1,2} stream only valid columns; the last
            # tile streams full width (its masked cols are zero in pt) so
            # every PSUM element's accumulation closes with stop=True.
            # The UNNORMALIZED O^T is cast straight into ot_sb (freeing the
            # po bank as soon as the recip-input ln also reads it);
            # normalization happens in place two pairs later.
            q0 = ci * 512
            njs = 4 * ci + 4
            pts = pts_map.pop((pr, ci))
            pos = []
            for half in range(2):
                h = 2 * pr + half
                fi, row = h // 2, (h % 2) * 64
                po = ps_o.tile([DH + 1, 512], F32, tag="o", name="po")
                pos.append(po)
                for j in range(njs):
                    r = j - 4 * ci
                    c0 = 128 * r if (1 <= r and j < njs - 1) else 0
                    nc.tensor.matmul(
                        po[:, c0:],
                        v_sb[j][:, h * VSTRIDE:h * VSTRIDE + DH + 1],
                        pts[j][:, half * 512 + c0:(half + 1) * 512],
                        start=(j == 0), stop=(j == njs - 1))
                nc.vector.tensor_copy(
                    ot_sb[fi][row:row + 64, q0:q0 + 512], po[0:DH, :])
            return [pr, ci, pos, None]

        def emit_recip(rec):
            # ln then exp(-x) of both sums rows (same ACT table set as the
            # softmax exps). Emitted inside the NEXT pair's exp stream so
            # the ACT never stalls waiting for the PV to finish.
            pr, ci, pos, _ = rec
            lrow = work.tile([1, 1024], F32, tag="lrow", name="lrow", bufs=2)
            rrow = work.tile([1, 1024], BF16, tag="rrow", name="rrow", bufs=2)
            for half in range(2):
                nc.scalar.activation(lrow[0:1, half * 512:(half + 1) * 512],
                                     pos[half][DH:DH + 1, :], Ln)
            nc.scalar.activation(rrow[:], lrow[:], Exp, scale=-1.0)
            rec[2] = None
            rec[3] = rrow

        def norm_finish(rec):
            # rank-1 matmuls broadcast each half's recip row across 64
            # partitions, then DVE scales O^T in place. Runs two pairs
            # after the PV, so the recip rows are always ready.
            pr, ci, _, rrow = rec
            q0 = ci * 512
            for half in range(2):
                h = 2 * pr + half
                fi, row = h // 2, (h % 2) * 64
                rb_ps = ps_mm.tile([DH, 512], F32, tag="mm", name="rb_ps")
                nc.tensor.matmul(
                    rb_ps[:], ones64[:],
                    rrow[0:1, half * 512:(half + 1) * 512],
                    start=True, stop=True)
                ot = ot_sb[fi][row:row + 64, q0:q0 + 512]
                nc.vector.tensor_mul(ot, ot, rb_ps[:])

        def emit_proj(ci, e0, e1):
            # projection for chunk ci's columns (all pairs' OT normalized)
            for ei in range(e0, e1):
                p = ps_mm.tile([128, 512], F32, tag="mm", name="p_proj")
                for fi in range(4):
                    nc.tensor.matmul(
                        p[:], wo_sb[fi][:, ei * 128:(ei + 1) * 128],
                        ot_sb[fi][:, ci * 512:(ci + 1) * 512],
                        start=(fi == 0), stop=(fi == 3))
                os_ = work.tile([128, 512], BF16, tag="os", name="os", bufs=2)
                nc.vector.tensor_copy(os_[:], p[:])
                eng = nc.sync if ei % 2 == 0 else nc.scalar
                eng.dma_start(
                    outT.ap()[ei * 128:(ei + 1) * 128,
                              ci * 512:(ci + 1) * 512], os_[:])

        # Two-phase woven schedule balancing PE-heavy projection work
        # against the ACT-bound exp stream. Phase 1: per head-pair pr, its
        # chunks 0..2, with the NEXT pair's Q/K chains woven in (they only
        # use the mm psum tag, so they slot into exp-paced PE bubbles and
        # the next block's sims start without a projection stall); phase
        # 2: the four chunk-3 pairs, PE-filled with V group 3 and the
        # deferred output projections. Within a pair: off-diagonal sims,
        # previous pair's recip (ACT), V fills, pair n-2's norm_finish,
        # projection fill, Q/K weave, diagonal sims, PV + casts.
        order = [(pr, ci) for pr in range(4) for ci in range(3)]
        order += [(pr, 3) for pr in range(4)]
        v_fill = {(0, 0): [0, 1, 2, 3], (0, 1): [4, 5, 6, 7],
                  (0, 2): [8, 9, 10, 11], (0, 3): [12, 13, 14, 15]}
        proj_fill = {(1, 3): 0, (2, 3): 1, (3, 3): 2}
        qk_fill = {(0, 0): ("q", 1), (0, 1): ("k", 1),
                   (1, 0): ("q", 2), (1, 1): ("k", 2),
                   (2, 0): ("q", 3), (2, 1): ("k", 3)}
        emit_qk("q", "wq", 0)
        emit_qk("k", "wk", 0)
        pipe = []   # records awaiting recip (last) / norm_finish (first)
        for pr, ci in order:
            emit_sim(pr, ci, 0, 4 * ci)
            if pipe and pipe[-1][3] is None:
                emit_recip(pipe[-1])
            for ti in v_fill.get((pr, ci), []):
                emit_v(ti)
            if len(pipe) >= 2:
                norm_finish(pipe.pop(0))
            if (pr, ci) in proj_fill:
                emit_proj(proj_fill[pr, ci], 0, 8)
            if (pr, ci) in qk_fill:
                qn, npr = qk_fill[pr, ci]
                emit_qk(qn, "wq" if qn == "q" else "wk", npr)
            emit_sim(pr, ci, 4 * ci, 4 * ci + 4)
            pipe.append(emit_pv(pr, ci))
        emit_recip(pipe[-1])
        norm_finish(pipe.pop(0))
        norm_finish(pipe.pop(0))
        emit_proj(NQC - 1, 0, 8)
    return nc


_NC = None


def _get_nc():
    global _NC
    if _NC is None:
        _patch_walrus_wait_limit()
        _NC = build_kernel()
    return _NC


def _host_masks():
    kl = np.arange(128)[:, None]
    ql = np.arange(512)[None, :]
    m = np.empty((4, 128, 512), dtype=ml_dtypes.bfloat16)
    for r in range(4):
        m[r] = (128 * r + kl <= ql).astype(np.float32)
    return m


def kernel(x, w_qkv, w_out, _trace=False, _trace_kwargs=None):
    x = np.asarray(x, dtype=np.float32)
    w_qkv = np.asarray(w_qkv, dtype=np.float32)
    w_out = np.asarray(w_out, dtype=np.float32)
    nc = _get_nc()

    msk = _host_masks()
    in_maps = []
    for c in range(NCORES):
        b, g = c // 2, c % 2
        cols = slice(g * FPC, (g + 1) * FPC)
        in_maps.append({
            "xT": np.ascontiguousarray(x[b].T).astype(ml_dtypes.bfloat16),
            "wq": w_qkv[:, 0 * DIM:1 * DIM][:, cols].astype(ml_dtypes.bfloat16),
            "wk": w_qkv[:, 1 * DIM:2 * DIM][:, cols].astype(ml_dtypes.bfloat16),
            "wv": w_qkv[:, 2 * DIM:3 * DIM][:, cols].astype(ml_dtypes.bfloat16),
            "wo": w_out[g * FPC:(g + 1) * FPC, :].astype(ml_dtypes.bfloat16),
            "msk": msk,
        })

    res = run_bass_kernel_spmd(
        nc, in_maps, core_ids=list(range(NCORES)),
        trace=_trace, **(_trace_kwargs or {}))
    out = np.empty((4, SEQ, DIM), dtype=np.float32)
    for b in range(4):
        out[b] = (res.results[2 * b]["outT"].astype(np.float32)
                  + res.results[2 * b + 1]["outT"].astype(np.float32)).T
    if _trace:
        kernel.last_results = res
    return out


# revision 20
# speedup vs baseline: 1.0142x; 1.0142x over previous
"""Causal MHA (batch=4, seq=2048, dim=1024, 16 heads x 64) on 8 TRN2 NeuronCores.

Sharding: core c handles batch b = c//2 and head-group g = c%2 (8 heads).
Each core computes QKV projections for its heads, causal attention, and a
partial output projection over its 512 features. The host sums the two
partial projections per batch and transposes back.

All matmuls run in bf16 (fp32 PSUM accumulate); softmax runs without max
subtraction (logits are bounded ~|8|), with the row sums produced by an
extra ones-column appended to V during the PV matmul.

Schedule: per (head-pair, q-chunk) the softmax-normalization chain
(ln/exp recip of the PV sums row, a rank-2 selector matmul that
broadcasts both halves' recip rows across 128 partitions, and the DVE
normalize-multiplies) is pipelined one pair behind, emitted between the
next pair's off-diagonal sims and its diagonal sims, so neither PE nor
the DVE mask-mul FIFO ever waits on it. Diagonal tiles stream only
their valid q columns through the PE on both the sim and PV matmuls.
"""
import sys

sys.path.insert(0, "/opt/trn_rl_repo")

import json
import numpy as np
import ml_dtypes
from contextlib import ExitStack

import concourse.bass as bass
import concourse.tile as tile
from concourse import mybir
from concourse import bass_utils as _bu
from concourse.bass_utils import run_bass_kernel_spmd

LDW_OPT = False  # walrus ldw-opt rejects bass-emitted Ldweights outright

BF16 = mybir.dt.bfloat16
F32 = mybir.dt.float32
F32R = mybir.dt.float32r
Exp = mybir.ActivationFunctionType.Exp
Ln = mybir.ActivationFunctionType.Ln

DIM = 1024
SEQ = 2048
NH = 16          # total heads
HPC = 8          # heads per core
DH = 64          # head dim
SCALE = DH ** -0.5
NCORES = 8
FPC = HPC * DH   # features per core = 512
NKT = SEQ // 128   # 16 k-tiles of 128
NQC = SEQ // 512   # 4 q-chunks of 512
VSTRIDE = DH + 2   # 66: V columns per head incl. ones col + pad

_WALRUS_PATCHED = False


def _patch_walrus_wait_limit():
    """This container's walrus rejects >1 sem wait per instruction
    (CoreV3 setupSyncWait). Tile's tail drain carries one wait per live
    proc; split the extras into preceding single-wait Drain carriers at
    BIR-JSON serialization time."""
    global _WALRUS_PATCHED
    if _WALRUS_PATCHED:
        return
    _WALRUS_PATCHED = True

    if LDW_OPT:
        orig_run = _bu.run_command

        def run_patched(cmd, *a, **k):
            cmd = ["--enable-ldw-opt=true" if c == "--enable-ldw-opt=false" else c
                   for c in cmd]
            return orig_run(cmd, *a, **k)

        _bu.run_command = run_patched

    orig = bass.Bass.to_json_bytes

    def _merge_ldw_halves(insts):
        """Fold row-tiled Ldweights pairs ([64,128] at row 0 + [64,128] at
        row 64 of the same tensor) into one [128,128] load carrying both
        halves' waits."""
        out = []
        pend = None  # (index_in_out, inst) of a candidate row-0 half
        for inst in insts:
            op = inst["opcode"]
            if inst.get("engine") != "PE":
                out.append(inst)
                continue
            if op == "Ldweights" and inst.get("tile_size") == [64, 128]:
                ap = inst["ins"][0].get("ap")
                if inst.get("tile_position") == [0, 0] and ap and ap[0][1] == 64:
                    out.append(inst)
                    pend = (len(out) - 1, inst)
                    continue
                if (pend is not None
                        and inst.get("tile_position") == [64, 0] and ap
                        and ap[0][1] == 64):
                    a = pend[1]
                    aap = a["ins"][0]["ap"]
                    same = (a["ins"][0].get("memref") == inst["ins"][0].get("memref")
                            and aap[0][0] == ap[0][0] and aap[1] == ap[1]
                            and inst["ins"][0].get("offset", 0)
                            == a["ins"][0].get("offset", 0) + 64 * aap[0][0])
                    b_si = inst.get("sync_info") or {}
                    if same and not b_si.get("on_update"):
                        aap[0][1] = 128
                        a["tile_size"] = [128, 128]
                        a.setdefault("sync_info", {"on_update": [], "on_wait": []})
                        a["sync_info"].setdefault("on_wait", [])
                        a["sync_info"]["on_wait"].extend(b_si.get("on_wait") or [])
                        pend = None
                        continue
                out.append(inst)
                pend = None
            else:
                if op not in ("Matmult", "NoOp"):
                    pend = None
                out.append(inst)
        return out

    def patched(self, *a, **k):
        d = json.loads(orig(self, *a, **k))
        for f in d["functions"]:
            for bb in f["blocks"]:
                bb["instructions"] = _merge_ldw_halves(bb["instructions"])
                out = []
                last_ldw = None  # (key, still_valid)
                for inst in bb["instructions"]:
                    si = inst.get("sync_info")
                    ow = (si or {}).get("on_wait") or []
                    op = inst["opcode"]

                    def emit_carriers(waits):
                        for j, w in enumerate(waits):
                            out.append({
                                "name": f"{inst['name']}__w{j}",
                                "opcode": "NoOp",
                                "engine": inst["engine"],
                                "ins": [], "outs": [],
                                "debug": inst.get("debug", 0),
                                "sync_info": {"on_update": [], "on_wait": [w]},
                            })

                    # drop a Ldweights identical to the previous one when only
                    # Matmult/NoOp sit between (weights already resident);
                    # also fold the row-tiled [64,128]+[64,128] half-pair into
                    # the single [128,128] load emitted by _merge_ldw_halves
                    if op == "Ldweights" and inst["engine"] == "PE":
                        key = json.dumps(
                            [inst.get("ins"), inst.get("tile_position"),
                             inst.get("tile_size")], sort_keys=True)
                        if last_ldw == key and not (si or {}).get("on_update"):
                            emit_carriers(ow)
                            continue
                        last_ldw = key
                    elif inst["engine"] == "PE" and op not in ("Matmult", "NoOp"):
                        last_ldw = None

                    if len(ow) > 1:
                        emit_carriers(ow[:-1])
                        si["on_wait"] = [ow[-1]]
                    out.append(inst)
                bb["instructions"] = out
        return json.dumps(d).encode()

    bass.Bass.to_json_bytes = patched


def build_kernel():
    nc = bass.Bass()
    xT = nc.declare_dram_parameter("xT", [DIM, SEQ], BF16, isOutput=False)
    wqkv = nc.declare_dram_parameter("wqkv", [DIM, 3 * FPC], BF16,
                                     isOutput=False)
    wo = nc.declare_dram_parameter("wo", [FPC, DIM], BF16, isOutput=False)
    # causal keep masks per diagonal offset r: [r, 128, 512]
    msk = nc.declare_dram_parameter("msk", [4, 128, 512], BF16, isOutput=False)
    outT = nc.declare_dram_parameter("outT", [DIM, SEQ], BF16, isOutput=True)

    with tile.TileContext(nc) as tc, ExitStack() as ctx:
        persist = ctx.enter_context(tc.tile_pool(name="persist", bufs=1))
        work = ctx.enter_context(tc.tile_pool(name="work", bufs=4))
        pt_pool = ctx.enter_context(tc.tile_pool(name="pt", bufs=1))
        ps_mm = ctx.enter_context(tc.tile_pool(name="ps_mm", bufs=2, space="PSUM"))
        ps_s = ctx.enter_context(tc.tile_pool(name="ps_s", bufs=2, space="PSUM"))
        ps_o = ctx.enter_context(tc.tile_pool(name="ps_o", bufs=2, space="PSUM"))

        # ---- load inputs. One batched DMA per tensor (xT in two halves):
        # a [di*128+p, c] HBM row block lands at SBUF [p, di*W + c], so a
        # per-di "tile" is a column slice of one wide tile. Few DMA issues
        # (the HWDGE ring serializes issue at ~0.6us each) and the first
        # QKV chain starts as soon as wq + the first xT half land. -------
        w_wide = persist.tile([128, 24 * FPC], BF16, tag="wqkv", name="wqkv")
        xT_wide = persist.tile([128, 8 * SEQ], BF16, tag="xT", name="xT")
        wo_wide = persist.tile([128, 4 * DIM], BF16, tag="wo", name="wo")
        msk_wide = persist.tile([128, 4 * 512], BF16, tag="msk", name="msk")
        _woff = {"wq": 0, "wk": FPC, "wv": 2 * FPC}

        def wsl(name, di):       # [128,FPC] view of weight block di
            o = di * 3 * FPC + _woff[name]
            return w_wide[:, o:o + FPC]

        def xsl(di):             # [128,SEQ] view of xT block di
            return xT_wide[:, di * SEQ:(di + 1) * SEQ]

        # weights on the scalar HWDGE ring, xT (in 4 chunks so the QKV
        # chains can start on the first k-quarter) on the sync ring —
        # a big DMA's transfer occupies its issuing queue, so two rings
        # let weights and activations stream in parallel.
        # xT in 4 chunks on the sync HWDGE ring; weights as per-di tiles
        # (contiguous HBM rows -> full DMA bandwidth, unlike the strided
        # batched form) split across the gpsimd SWDGE and scalar HWDGE
        # queues so issue serialization doesn't delay late tensors.
        for qt in range(4):
            nc.sync.dma_start(
                xT_wide[:].rearrange("p (di c) -> p di c", di=8)[:, 2 * qt:2 * qt + 2],
                xT.ap().rearrange("(di p) c -> p di c", di=8)[:, 2 * qt:2 * qt + 2])
        for di in range(8):
            nc.gpsimd.dma_start(
                w_wide[:, di * 3 * FPC:(di + 1) * 3 * FPC],
                wqkv.ap()[di * 128:(di + 1) * 128, :])
        nc.scalar.dma_start(
            wo_wide[:].rearrange("p (fi c) -> p fi c", fi=4),
            wo.ap().rearrange("(fi p) c -> p fi c", fi=4))
        nc.scalar.dma_start(
            msk_wide[:].rearrange("p (r c) -> p r c", r=4),
            msk.ap().rearrange("r p c -> p r c"))
        wo_sb = [wo_wide[:, fi * DIM:(fi + 1) * DIM] for fi in range(4)]
        msk_sb = [msk_wide[:, r * 512:(r + 1) * 512] for r in range(4)]
        ones64 = persist.tile([1, DH], BF16, tag="ones64")
        nc.gpsimd.memset(ones64[:], 1.0)

        # ---- stage B: QKV projections -----------------------------------
        qk_sb = {"q": [], "k": []}
        for qn in ("q", "k"):
            for fi in range(4):
                qk_sb[qn].append(
                    persist.tile([128, SEQ], BF16, tag=f"{qn}{fi}",
                                 name=f"{qn}{fi}"))
        v_sb = [persist.tile([128, HPC * VSTRIDE], BF16, tag=f"v{ti}",
                             name=f"v{ti}") for ti in range(NKT)]

        def chains4():
            # 4 simultaneous [128,512] accumulators: 2 from the mm tag plus 2
            # borrowed from the (momentarily idle) o tag — keeps weights
            # stationary across 4 matmuls so the LDW dedupe can drop 3 of 4
            return [ps_mm.tile([128, 512], F32, tag="mm", name="ch0"),
                    ps_mm.tile([128, 512], F32, tag="mm", name="ch1"),
                    ps_o.tile([128, 512], F32, tag="o", name="ch2"),
                    ps_o.tile([128, 512], F32, tag="o", name="ch3")]

        def emit_qk(qn, wn, fi):
            # Q, K in [feature, token] layout (w stationary, xT moving)
            t = qk_sb[qn][fi]
            ch = chains4()
            for di in range(8):
                for tck in range(4):
                    nc.tensor.matmul(
                        ch[tck][:], wsl(wn, di)[:, fi * 128:(fi + 1) * 128],
                        xsl(di)[:, tck * 512:(tck + 1) * 512],
                        start=(di == 0), stop=(di == 7))
            for tck in range(4):
                nc.vector.tensor_copy(t[:, tck * 512:(tck + 1) * 512], ch[tck][:])

        def emit_v(ti):
            # V in [token, feature] layout (xT stationary, wv moving), strided
            # into VSTRIDE-blocks with a ones column per head
            t = v_sb[ti]
            p = ps_mm.tile([128, 512], F32, tag="mm", name="p_v")
            for di in range(8):
                nc.tensor.matmul(
                    p[:], xsl(di)[:, ti * 128:(ti + 1) * 128],
                    wsl("wv", di),
                    start=(di == 0), stop=(di == 7))
            dst = t[:].rearrange("p (h c) -> p h c", h=HPC)[:, :, 0:DH]
            src = p[:].rearrange("p (h c) -> p h c", h=HPC)
            nc.vector.tensor_copy(dst, src)
            nc.gpsimd.memset(
                t[:].rearrange("p (h c) -> p h c", h=HPC)[:, :, DH:DH + 1], 1.0)

        ot_sb = [persist.tile([128, SEQ], BF16, tag=f"ot{fi}", name=f"ot{fi}")
                 for fi in range(4)]
        pts_map = {}

        def emit_sim(pr, ci, j0, j1):
            # S^T strips + exp into pt tiles for (head pair pr, q-chunk ci),
            # k-tiles j0..j1-1. Diagonal tiles (r >= 1) stream only their
            # valid q columns.
            q0 = ci * 512
            pts = pts_map.setdefault((pr, ci), [])
            for j in range(j0, j1):
                r = j - 4 * ci
                c0 = 128 * r if r > 0 else 0   # first valid q col in chunk
                ps = ps_s.tile([128, 1024], F32, tag="s", name="ps_st")
                for half in range(2):   # head A / head B, row-tiled
                    nc.tensor.matmul(
                        ps[:, half * 512 + c0:(half + 1) * 512],
                        qk_sb["k"][pr][half * 64:(half + 1) * 64,
                                       j * 128:(j + 1) * 128],
                        qk_sb["q"][pr][half * 64:(half + 1) * 64,
                                       q0 + c0:q0 + 512],
                        start=True, stop=True)
                pt = pt_pool.tile([128, 1024], BF16, tag=f"pt{j}", name="pt",
                                  bufs=2 if j < 8 else 1)
                pts.append(pt)
                if r < 0:
                    nc.scalar.activation(pt[:], ps[:], Exp, scale=SCALE)
                else:
                    # diagonal tile: columns ql >= 128r are valid; the
                    # rest must be zero (PV streams the full chunk on its
                    # closing matmul)
                    pt3 = pt[:].rearrange("p (b w) -> p b w", b=2)[:, :, c0:]
                    ps3 = ps[:].rearrange("p (b w) -> p b w", b=2)[:, :, c0:]
                    m3 = msk_sb[r][:, c0:][:, None, :].broadcast_to(
                        [128, 2, 512 - c0])
                    if r > 0:
                        nc.gpsimd.memset(
                            pt[:].rearrange("p (b w) -> p b w", b=2)[:, :, 0:c0],
                            0.0)
                    nc.scalar.activation(pt3, ps3, Exp, scale=SCALE)
                    nc.vector.tensor_mul(pt3, pt3, m3)

        def emit_pv(pr, ci):
            # PV: V_aug stationary [128k, 65], P^T moving.
            # Output O^T_aug [65, 512q]: rows 0:64 = O^T, row 64 = sums.
            # Diagonal tiles r in {1,2} stream only valid columns; the last
            # tile streams full width (its masked cols are zero in pt) so
            # every PSUM element's accumulation closes with stop=True.
            # The UNNORMALIZED O^T is cast straight into ot_sb (freeing the
            # po bank as soon as the recip-input ln also reads it);
            # normalization happens in place two pairs later.
            q0 = ci * 512
            njs = 4 * ci + 4
            pts = pts_map.pop((pr, ci))
            pos = []
            for half in range(2):
                h = 2 * pr + half
                fi, row = h // 2, (h % 2) * 64
                po = ps_o.tile([DH + 1, 512], F32, tag="o", name="po")
                pos.append(po)
                for j in range(njs):
                    r = j - 4 * ci
                    c0 = 128 * r if (1 <= r and j < njs - 1) else 0
                    nc.tensor.matmul(
                        po[:, c0:],
                        v_sb[j][:, h * VSTRIDE:h * VSTRIDE + DH + 1],
                        pts[j][:, half * 512 + c0:(half + 1) * 512],
                        start=(j == 0), stop=(j == njs - 1))
                nc.vector.tensor_copy(
                    ot_sb[fi][row:row + 64, q0:q0 + 512], po[0:DH, :])
            return [pr, ci, pos, None]

        def emit_recip(rec):
            # ln then exp(-x) of both sums rows (same ACT table set as the
            # softmax exps). Emitted inside the NEXT pair's exp stream so
            # the ACT never stalls waiting for the PV to finish.
            pr, ci, pos, _ = rec
            lrow = work.tile([1, 1024], F32, tag="lrow", name="lrow", bufs=2)
            rrow = work.tile([1, 1024], BF16, tag="rrow", name="rrow", bufs=2)
            for half in range(2):
                nc.scalar.activation(lrow[0:1, half * 512:(half + 1) * 512],
                                     pos[half][DH:DH + 1, :], Ln)
            nc.scalar.activation(rrow[:], lrow[:], Exp, scale=-1.0)
            rec[2] = None
            rec[3] = rrow

        def norm_finish(rec):
            # rank-1 matmuls broadcast each half's recip row across 64
            # partitions, then DVE scales O^T in place. Runs two pairs
            # after the PV, so the recip rows are always ready.
            pr, ci, _, rrow = rec
            q0 = ci * 512
            for half in range(2):
                h = 2 * pr + half
                fi, row = h // 2, (h % 2) * 64
                rb_ps = ps_mm.tile([DH, 512], F32, tag="mm", name="rb_ps")
                nc.tensor.matmul(
                    rb_ps[:], ones64[:],
                    rrow[0:1, half * 512:(half + 1) * 512],
                    start=True, stop=True)
                ot = ot_sb[fi][row:row + 64, q0:q0 + 512]
                nc.vector.tensor_mul(ot, ot, rb_ps[:])

        def emit_proj(ci, e0, e1):
            # projection for chunk ci's columns (all pairs' OT normalized)
            for ei in range(e0, e1):
                p = ps_mm.tile([128, 512], F32, tag="mm", name="p_proj")
                for fi in range(4):
                    nc.tensor.matmul(
                        p[:], wo_sb[fi][:, ei * 128:(ei + 1) * 128],
                        ot_sb[fi][:, ci * 512:(ci + 1) * 512],
                        start=(fi == 0), stop=(fi == 3))
                os_ = work.tile([128, 512], BF16, tag="os", name="os", bufs=2)
                nc.vector.tensor_copy(os_[:], p[:])
                eng = nc.sync if ei % 2 == 0 else nc.scalar
                eng.dma_start(
                    outT.ap()[ei * 128:(ei + 1) * 128,
                              ci * 512:(ci + 1) * 512], os_[:])

        # Two-phase woven schedule balancing PE-heavy projection work
        # against the ACT-bound exp stream. Phase 1: per head-pair pr, its
        # chunks 0..2, with the NEXT pair's Q/K chains woven in (they only
        # use the mm psum tag, so they slot into exp-paced PE bubbles and
        # the next block's sims start without a projection stall); phase
        # 2: the four chunk-3 pairs, PE-filled with V group 3 and the
        # deferred output projections. Within a pair: off-diagonal sims,
        # previous pair's recip (ACT), V fills, pair n-2's norm_finish,
        # projection fill, Q/K weave, diagonal sims, PV + casts.
        order = [(pr, ci) for pr in range(4) for ci in range(3)]
        order += [(pr, 3) for pr in range(4)]
        v_fill = {(0, 0): [0, 1, 2, 3], (0, 1): [4, 5, 6, 7],
                  (0, 2): [8, 9, 10, 11], (0, 3): [12, 13, 14, 15]}
        proj_fill = {(1, 3): 0, (2, 3): 1, (3, 3): 2}
        # next block's Q/K chains, emitted right after this pair's PV so
        # the next block's sims never wait on them; chains4 borrows ps_o,
        # so the pending recip (reading the last po) must flush first.
        qk_post = {(0, 0): ("q", 1), (0, 1): ("k", 1),
                   (1, 0): ("q", 2), (1, 1): ("k", 2),
                   (2, 0): ("q", 3), (2, 1): ("k", 3)}
        emit_qk("q", "wq", 0)
        emit_qk("k", "wk", 0)
        pipe = []   # records awaiting recip (last) / norm_finish (first)
        for pr, ci in order:
            emit_sim(pr, ci, 0, 4 * ci)
            if pipe and pipe[-1][3] is None:
                emit_recip(pipe[-1])
            for ti in v_fill.get((pr, ci), []):
                emit_v(ti)
            if len(pipe) >= 2:
                norm_finish(pipe.pop(0))
            if (pr, ci) in proj_fill:
                emit_proj(proj_fill[pr, ci], 0, 8)
            emit_sim(pr, ci, 4 * ci, 4 * ci + 4)
            pipe.append(emit_pv(pr, ci))
            if (pr, ci) in qk_post:
                emit_recip(pipe[-1])
                qn, npr = qk_post[pr, ci]
                emit_qk(qn, "wq" if qn == "q" else "wk", npr)
        emit_recip(pipe[-1])
        norm_finish(pipe.pop(0))
        norm_finish(pipe.pop(0))
        emit_proj(NQC - 1, 0, 8)
    return nc


_NC = None


def _get_nc():
    global _NC
    if _NC is None:
        _patch_walrus_wait_limit()
        _NC = build_kernel()
    return _NC


def _host_masks():
    kl = np.arange(128)[:, None]
    ql = np.arange(512)[None, :]
    m = np.empty((4, 128, 512), dtype=ml_dtypes.bfloat16)
    for r in range(4):
        m[r] = (128 * r + kl <= ql).astype(np.float32)
    return m


def kernel(x, w_qkv, w_out, _trace=False, _trace_kwargs=None):
    x = np.asarray(x, dtype=np.float32)
    w_qkv = np.asarray(w_qkv, dtype=np.float32)
    w_out = np.asarray(w_out, dtype=np.float32)
    nc = _get_nc()

    msk = _host_masks()
    in_maps = []
    for c in range(NCORES):
        b, g = c // 2, c % 2
        cols = slice(g * FPC, (g + 1) * FPC)
        wq_ = w_qkv[:, 0 * DIM:1 * DIM][:, cols]
        wk_ = w_qkv[:, 1 * DIM:2 * DIM][:, cols]
        wv_ = w_qkv[:, 2 * DIM:3 * DIM][:, cols]
        in_maps.append({
            "xT": np.ascontiguousarray(x[b].T).astype(ml_dtypes.bfloat16),
            "wqkv": np.concatenate([wq_, wk_, wv_], axis=1)
                      .astype(ml_dtypes.bfloat16),
            "wo": w_out[g * FPC:(g + 1) * FPC, :].astype(ml_dtypes.bfloat16),
            "msk": msk,
        })

    res = run_bass_kernel_spmd(
        nc, in_maps, core_ids=list(range(NCORES)),
        trace=_trace, **(_trace_kwargs or {}))
    out = np.empty((4, SEQ, DIM), dtype=np.float32)
    for b in range(4):
        out[b] = (res.results[2 * b]["outT"].astype(np.float32)
                  + res.results[2 * b + 1]["outT"].astype(np.float32)).T
    if _trace:
        kernel.last_results = res
    return out


# revision 21
# speedup vs baseline: 1.0176x; 1.0033x over previous
"""Causal MHA (batch=4, seq=2048, dim=1024, 16 heads x 64) on 8 TRN2 NeuronCores.

Sharding: core c handles batch b = c//2 and head-group g = c%2 (8 heads).
Each core computes QKV projections for its heads, causal attention, and a
partial output projection over its 512 features. The host sums the two
partial projections per batch and transposes back.

All matmuls run in bf16 (fp32 PSUM accumulate); softmax runs without max
subtraction (logits are bounded ~|8|), with the row sums produced by an
extra ones-column appended to V during the PV matmul.

Schedule: per (head-pair, q-chunk) the softmax-normalization chain
(ln/exp recip of the PV sums row, a rank-2 selector matmul that
broadcasts both halves' recip rows across 128 partitions, and the DVE
normalize-multiplies) is pipelined one pair behind, emitted between the
next pair's off-diagonal sims and its diagonal sims, so neither PE nor
the DVE mask-mul FIFO ever waits on it. Diagonal tiles stream only
their valid q columns through the PE on both the sim and PV matmuls.
"""
import sys

sys.path.insert(0, "/opt/trn_rl_repo")

import json
import numpy as np
import ml_dtypes
from contextlib import ExitStack

import concourse.bass as bass
import concourse.tile as tile
from concourse import mybir
from concourse import bass_utils as _bu
from concourse.bass_utils import run_bass_kernel_spmd

LDW_OPT = False  # walrus ldw-opt rejects bass-emitted Ldweights outright

BF16 = mybir.dt.bfloat16
F32 = mybir.dt.float32
F32R = mybir.dt.float32r
Exp = mybir.ActivationFunctionType.Exp
Ln = mybir.ActivationFunctionType.Ln

DIM = 1024
SEQ = 2048
NH = 16          # total heads
HPC = 8          # heads per core
DH = 64          # head dim
SCALE = DH ** -0.5
NCORES = 8
FPC = HPC * DH   # features per core = 512
NKT = SEQ // 128   # 16 k-tiles of 128
NQC = SEQ // 512   # 4 q-chunks of 512
VSTRIDE = DH + 2   # 66: V columns per head incl. ones col + pad

_WALRUS_PATCHED = False


def _patch_walrus_wait_limit():
    """This container's walrus rejects >1 sem wait per instruction
    (CoreV3 setupSyncWait). Tile's tail drain carries one wait per live
    proc; split the extras into preceding single-wait Drain carriers at
    BIR-JSON serialization time."""
    global _WALRUS_PATCHED
    if _WALRUS_PATCHED:
        return
    _WALRUS_PATCHED = True

    if LDW_OPT:
        orig_run = _bu.run_command

        def run_patched(cmd, *a, **k):
            cmd = ["--enable-ldw-opt=true" if c == "--enable-ldw-opt=false" else c
                   for c in cmd]
            return orig_run(cmd, *a, **k)

        _bu.run_command = run_patched

    orig = bass.Bass.to_json_bytes

    def _merge_ldw_halves(insts):
        """Fold row-tiled Ldweights pairs ([64,128] at row 0 + [64,128] at
        row 64 of the same tensor) into one [128,128] load carrying both
        halves' waits."""
        out = []
        pend = None  # (index_in_out, inst) of a candidate row-0 half
        for inst in insts:
            op = inst["opcode"]
            if inst.get("engine") != "PE":
                out.append(inst)
                continue
            if op == "Ldweights" and inst.get("tile_size") == [64, 128]:
                ap = inst["ins"][0].get("ap")
                if inst.get("tile_position") == [0, 0] and ap and ap[0][1] == 64:
                    out.append(inst)
                    pend = (len(out) - 1, inst)
                    continue
                if (pend is not None
                        and inst.get("tile_position") == [64, 0] and ap
                        and ap[0][1] == 64):
                    a = pend[1]
                    aap = a["ins"][0]["ap"]
                    same = (a["ins"][0].get("memref") == inst["ins"][0].get("memref")
                            and aap[0][0] == ap[0][0] and aap[1] == ap[1]
                            and inst["ins"][0].get("offset", 0)
                            == a["ins"][0].get("offset", 0) + 64 * aap[0][0])
                    b_si = inst.get("sync_info") or {}
                    if same and not b_si.get("on_update"):
                        aap[0][1] = 128
                        a["tile_size"] = [128, 128]
                        a.setdefault("sync_info", {"on_update": [], "on_wait": []})
                        a["sync_info"].setdefault("on_wait", [])
                        a["sync_info"]["on_wait"].extend(b_si.get("on_wait") or [])
                        pend = None
                        continue
                out.append(inst)
                pend = None
            else:
                if op not in ("Matmult", "NoOp"):
                    pend = None
                out.append(inst)
        return out

    def patched(self, *a, **k):
        d = json.loads(orig(self, *a, **k))
        for f in d["functions"]:
            for bb in f["blocks"]:
                bb["instructions"] = _merge_ldw_halves(bb["instructions"])
                out = []
                last_ldw = None  # (key, still_valid)
                for inst in bb["instructions"]:
                    si = inst.get("sync_info")
                    ow = (si or {}).get("on_wait") or []
                    op = inst["opcode"]

                    def emit_carriers(waits):
                        for j, w in enumerate(waits):
                            out.append({
                                "name": f"{inst['name']}__w{j}",
                                "opcode": "NoOp",
                                "engine": inst["engine"],
                                "ins": [], "outs": [],
                                "debug": inst.get("debug", 0),
                                "sync_info": {"on_update": [], "on_wait": [w]},
                            })

                    # drop a Ldweights identical to the previous one when only
                    # Matmult/NoOp sit between (weights already resident);
                    # also fold the row-tiled [64,128]+[64,128] half-pair into
                    # the single [128,128] load emitted by _merge_ldw_halves
                    if op == "Ldweights" and inst["engine"] == "PE":
                        key = json.dumps(
                            [inst.get("ins"), inst.get("tile_position"),
                             inst.get("tile_size")], sort_keys=True)
                        if last_ldw == key and not (si or {}).get("on_update"):
                            emit_carriers(ow)
                            continue
                        last_ldw = key
                    elif inst["engine"] == "PE" and op not in ("Matmult", "NoOp"):
                        last_ldw = None

                    if len(ow) > 1:
                        emit_carriers(ow[:-1])
                        si["on_wait"] = [ow[-1]]
                    out.append(inst)
                bb["instructions"] = out
        return json.dumps(d).encode()

    bass.Bass.to_json_bytes = patched


def build_kernel():
    nc = bass.Bass()
    xT = nc.declare_dram_parameter("xT", [DIM, SEQ], BF16, isOutput=False)
    wqkv = nc.declare_dram_parameter("wqkv", [DIM, 3 * FPC], BF16,
                                     isOutput=False)
    wo = nc.declare_dram_parameter("wo", [FPC, DIM], BF16, isOutput=False)
    # tri[:, 0:128] = strict upper-triangular -30000 (causal mask addend,
    # lhsT orientation); tri[:, 128:256] = identity (its moving operand)
    tri = nc.declare_dram_parameter("tri", [128, 256], BF16, isOutput=False)
    outT = nc.declare_dram_parameter("outT", [DIM, SEQ], BF16, isOutput=True)

    with tile.TileContext(nc) as tc, ExitStack() as ctx:
        persist = ctx.enter_context(tc.tile_pool(name="persist", bufs=1))
        work = ctx.enter_context(tc.tile_pool(name="work", bufs=4))
        pt_pool = ctx.enter_context(tc.tile_pool(name="pt", bufs=1))
        ps_mm = ctx.enter_context(tc.tile_pool(name="ps_mm", bufs=2, space="PSUM"))
        ps_s = ctx.enter_context(tc.tile_pool(name="ps_s", bufs=2, space="PSUM"))
        ps_o = ctx.enter_context(tc.tile_pool(name="ps_o", bufs=2, space="PSUM"))

        # ---- load inputs. One batched DMA per tensor (xT in two halves):
        # a [di*128+p, c] HBM row block lands at SBUF [p, di*W + c], so a
        # per-di "tile" is a column slice of one wide tile. Few DMA issues
        # (the HWDGE ring serializes issue at ~0.6us each) and the first
        # QKV chain starts as soon as wq + the first xT half land. -------
        w_wide = persist.tile([128, 24 * FPC], BF16, tag="wqkv", name="wqkv")
        xT_wide = persist.tile([128, 8 * SEQ], BF16, tag="xT", name="xT")
        wo_wide = persist.tile([128, 4 * DIM], BF16, tag="wo", name="wo")
        tri_sb = persist.tile([128, 256], BF16, tag="tri", name="tri")
        _woff = {"wq": 0, "wk": FPC, "wv": 2 * FPC}

        def wsl(name, di):       # [128,FPC] view of weight block di
            o = di * 3 * FPC + _woff[name]
            return w_wide[:, o:o + FPC]

        def xsl(di):             # [128,SEQ] view of xT block di
            return xT_wide[:, di * SEQ:(di + 1) * SEQ]

        # weights on the scalar HWDGE ring, xT (in 4 chunks so the QKV
        # chains can start on the first k-quarter) on the sync ring —
        # a big DMA's transfer occupies its issuing queue, so two rings
        # let weights and activations stream in parallel.
        # xT in 4 chunks on the sync HWDGE ring; weights as per-di tiles
        # (contiguous HBM rows -> full DMA bandwidth, unlike the strided
        # batched form) split across the gpsimd SWDGE and scalar HWDGE
        # queues so issue serialization doesn't delay late tensors.
        for qt in range(4):
            nc.sync.dma_start(
                xT_wide[:].rearrange("p (di c) -> p di c", di=8)[:, 2 * qt:2 * qt + 2],
                xT.ap().rearrange("(di p) c -> p di c", di=8)[:, 2 * qt:2 * qt + 2])
        for di in range(8):
            nc.gpsimd.dma_start(
                w_wide[:, di * 3 * FPC:(di + 1) * 3 * FPC],
                wqkv.ap()[di * 128:(di + 1) * 128, :])
        nc.scalar.dma_start(
            wo_wide[:].rearrange("p (fi c) -> p fi c", fi=4),
            wo.ap().rearrange("(fi p) c -> p fi c", fi=4))
        nc.scalar.dma_start(tri_sb[:], tri.ap())
        wo_sb = [wo_wide[:, fi * DIM:(fi + 1) * DIM] for fi in range(4)]
        ones64 = persist.tile([1, DH], BF16, tag="ones64")
        nc.gpsimd.memset(ones64[:], 1.0)

        # ---- stage B: QKV projections -----------------------------------
        qk_sb = {"q": [], "k": []}
        for qn in ("q", "k"):
            for fi in range(4):
                qk_sb[qn].append(
                    persist.tile([128, SEQ], BF16, tag=f"{qn}{fi}",
                                 name=f"{qn}{fi}"))
        v_sb = [persist.tile([128, HPC * VSTRIDE], BF16, tag=f"v{ti}",
                             name=f"v{ti}") for ti in range(NKT)]

        def chains4():
            # 4 simultaneous [128,512] accumulators: 2 from the mm tag plus 2
            # borrowed from the (momentarily idle) o tag — keeps weights
            # stationary across 4 matmuls so the LDW dedupe can drop 3 of 4
            return [ps_mm.tile([128, 512], F32, tag="mm", name="ch0"),
                    ps_mm.tile([128, 512], F32, tag="mm", name="ch1"),
                    ps_o.tile([128, 512], F32, tag="o", name="ch2"),
                    ps_o.tile([128, 512], F32, tag="o", name="ch3")]

        def emit_qk(qn, wn, fi):
            # Q, K in [feature, token] layout (w stationary, xT moving)
            t = qk_sb[qn][fi]
            ch = chains4()
            for di in range(8):
                for tck in range(4):
                    nc.tensor.matmul(
                        ch[tck][:], wsl(wn, di)[:, fi * 128:(fi + 1) * 128],
                        xsl(di)[:, tck * 512:(tck + 1) * 512],
                        start=(di == 0), stop=(di == 7))
            for tck in range(4):
                nc.vector.tensor_copy(t[:, tck * 512:(tck + 1) * 512], ch[tck][:])

        def emit_v(ti):
            # V in [token, feature] layout (xT stationary, wv moving), strided
            # into VSTRIDE-blocks with a ones column per head
            t = v_sb[ti]
            p = ps_mm.tile([128, 512], F32, tag="mm", name="p_v")
            for di in range(8):
                nc.tensor.matmul(
                    p[:], xsl(di)[:, ti * 128:(ti + 1) * 128],
                    wsl("wv", di),
                    start=(di == 0), stop=(di == 7))
            dst = t[:].rearrange("p (h c) -> p h c", h=HPC)[:, :, 0:DH]
            src = p[:].rearrange("p (h c) -> p h c", h=HPC)
            nc.vector.tensor_copy(dst, src)
            nc.gpsimd.memset(
                t[:].rearrange("p (h c) -> p h c", h=HPC)[:, :, DH:DH + 1], 1.0)

        ot_sb = [persist.tile([128, SEQ], BF16, tag=f"ot{fi}", name=f"ot{fi}")
                 for fi in range(4)]
        pts_map = {}

        def emit_sim(pr, ci, j0, j1):
            # S^T strips + exp into pt tiles for (head pair pr, q-chunk ci),
            # k-tiles j0..j1-1. Diagonal tiles (r >= 1) stream only their
            # valid q columns.
            q0 = ci * 512
            pts = pts_map.setdefault((pr, ci), [])
            for j in range(j0, j1):
                r = j - 4 * ci
                c0 = 128 * r if r > 0 else 0   # first valid q col in chunk
                ps = ps_s.tile([128, 1024], F32, tag="s", name="ps_st")
                for half in range(2):   # head A / head B, row-tiled
                    nc.tensor.matmul(
                        ps[:, half * 512 + c0:(half + 1) * 512],
                        qk_sb["k"][pr][half * 64:(half + 1) * 64,
                                       j * 128:(j + 1) * 128],
                        qk_sb["q"][pr][half * 64:(half + 1) * 64,
                                       q0 + c0:q0 + 512],
                        start=True, stop=True)
                pt = pt_pool.tile([128, 1024], BF16, tag=f"pt{j}", name="pt",
                                  bufs=2 if j < 8 else 1)
                pts.append(pt)
                if r < 0:
                    nc.scalar.activation(pt[:], ps[:], Exp, scale=SCALE)
                else:
                    # diagonal tile: add -30000 above the in-block diagonal
                    # (exp then underflows to exactly 0), zero the columns
                    # left of the valid range (PV streams the full chunk on
                    # its closing matmul)
                    for half in range(2):
                        nc.tensor.matmul(
                            ps[:, half * 512 + c0:half * 512 + c0 + 128],
                            tri_sb[:, 0:128], tri_sb[:, 128:256],
                            start=False, stop=True)
                    pt3 = pt[:].rearrange("p (b w) -> p b w", b=2)[:, :, c0:]
                    ps3 = ps[:].rearrange("p (b w) -> p b w", b=2)[:, :, c0:]
                    if r > 0:
                        nc.gpsimd.memset(
                            pt[:].rearrange("p (b w) -> p b w", b=2)[:, :, 0:c0],
                            0.0)
                    nc.scalar.activation(pt3, ps3, Exp, scale=SCALE)

        def emit_pv(pr, ci):
            # PV: V_aug stationary [128k, 65], P^T moving.
            # Output O^T_aug [65, 512q]: rows 0:64 = O^T, row 64 = sums.
            # Diagonal tiles r in {1,2} stream only valid columns; the last
            # tile streams full width (its masked cols are zero in pt) so
            # every PSUM element's accumulation closes with stop=True.
            # The UNNORMALIZED O^T is cast straight into ot_sb (freeing the
            # po bank as soon as the recip-input ln also reads it);
            # normalization happens in place two pairs later.
            q0 = ci * 512
            njs = 4 * ci + 4
            pts = pts_map.pop((pr, ci))
            pos = []
            for half in range(2):
                h = 2 * pr + half
                fi, row = h // 2, (h % 2) * 64
                po = ps_o.tile([DH + 1, 512], F32, tag="o", name="po")
                pos.append(po)
                for j in range(njs):
                    r = j - 4 * ci
                    c0 = 128 * r if (1 <= r and j < njs - 1) else 0
                    nc.tensor.matmul(
                        po[:, c0:],
                        v_sb[j][:, h * VSTRIDE:h * VSTRIDE + DH + 1],
                        pts[j][:, half * 512 + c0:(half + 1) * 512],
                        start=(j == 0), stop=(j == njs - 1))
                nc.vector.tensor_copy(
                    ot_sb[fi][row:row + 64, q0:q0 + 512], po[0:DH, :])
            return [pr, ci, pos, None]

        def emit_recip(rec):
            # ln then exp(-x) of both sums rows (same ACT table set as the
            # softmax exps). Emitted inside the NEXT pair's exp stream so
            # the ACT never stalls waiting for the PV to finish.
            pr, ci, pos, _ = rec
            lrow = work.tile([1, 1024], F32, tag="lrow", name="lrow", bufs=2)
            rrow = work.tile([1, 1024], BF16, tag="rrow", name="rrow", bufs=2)
            for half in range(2):
                nc.scalar.activation(lrow[0:1, half * 512:(half + 1) * 512],
                                     pos[half][DH:DH + 1, :], Ln)
            nc.scalar.activation(rrow[:], lrow[:], Exp, scale=-1.0)
            rec[2] = None
            rec[3] = rrow

        def norm_finish(rec):
            # rank-1 matmuls broadcast each half's recip row across 64
            # partitions, then DVE scales O^T in place. Runs two pairs
            # after the PV, so the recip rows are always ready.
            pr, ci, _, rrow = rec
            q0 = ci * 512
            for half in range(2):
                h = 2 * pr + half
                fi, row = h // 2, (h % 2) * 64
                rb_ps = ps_mm.tile([DH, 512], F32, tag="mm", name="rb_ps")
                nc.tensor.matmul(
                    rb_ps[:], ones64[:],
                    rrow[0:1, half * 512:(half + 1) * 512],
                    start=True, stop=True)
                ot = ot_sb[fi][row:row + 64, q0:q0 + 512]
                nc.vector.tensor_mul(ot, ot, rb_ps[:])

        def emit_proj(ci, e0, e1):
            # projection for chunk ci's columns (all pairs' OT normalized)
            for ei in range(e0, e1):
                p = ps_mm.tile([128, 512], F32, tag="mm", name="p_proj")
                for fi in range(4):
                    nc.tensor.matmul(
                        p[:], wo_sb[fi][:, ei * 128:(ei + 1) * 128],
                        ot_sb[fi][:, ci * 512:(ci + 1) * 512],
                        start=(fi == 0), stop=(fi == 3))
                os_ = work.tile([128, 512], BF16, tag="os", name="os", bufs=2)
                nc.vector.tensor_copy(os_[:], p[:])
                eng = nc.sync if ei % 2 == 0 else nc.scalar
                eng.dma_start(
                    outT.ap()[ei * 128:(ei + 1) * 128,
                              ci * 512:(ci + 1) * 512], os_[:])

        # Two-phase woven schedule balancing PE-heavy projection work
        # against the ACT-bound exp stream. Phase 1: per head-pair pr, its
        # chunks 0..2, with the NEXT pair's Q/K chains woven in (they only
        # use the mm psum tag, so they slot into exp-paced PE bubbles and
        # the next block's sims start without a projection stall); phase
        # 2: the four chunk-3 pairs, PE-filled with V group 3 and the
        # deferred output projections. Within a pair: off-diagonal sims,
        # previous pair's recip (ACT), V fills, pair n-2's norm_finish,
        # projection fill, Q/K weave, diagonal sims, PV + casts.
        order = [(pr, ci) for pr in range(4) for ci in range(3)]
        order += [(pr, 3) for pr in range(4)]
        v_fill = {(0, 0): [0, 1, 2, 3], (0, 1): [4, 5, 6, 7],
                  (0, 2): [8, 9, 10, 11], (0, 3): [12, 13, 14, 15]}
        proj_fill = {(1, 3): 0, (2, 3): 1, (3, 3): 2}
        # next block's Q/K chains, emitted right after this pair's PV so
        # the next block's sims never wait on them; chains4 borrows ps_o,
        # so the pending recip (reading the last po) must flush first.
        qk_post = {(0, 0): ("q", 1), (0, 1): ("k", 1),
                   (1, 0): ("q", 2), (1, 1): ("k", 2),
                   (2, 0): ("q", 3), (2, 1): ("k", 3)}
        emit_qk("q", "wq", 0)
        emit_qk("k", "wk", 0)
        pipe = []   # records awaiting recip (last) / norm_finish (first)
        for pr, ci in order:
            emit_sim(pr, ci, 0, 4 * ci)
            if pipe and pipe[-1][3] is None:
                emit_recip(pipe[-1])
            for ti in v_fill.get((pr, ci), []):
                emit_v(ti)
            if len(pipe) >= 2:
                norm_finish(pipe.pop(0))
            if (pr, ci) in proj_fill:
                emit_proj(proj_fill[pr, ci], 0, 8)
            emit_sim(pr, ci, 4 * ci, 4 * ci + 4)
            pipe.append(emit_pv(pr, ci))
            if (pr, ci) in qk_post:
                emit_recip(pipe[-1])
                qn, npr = qk_post[pr, ci]
                emit_qk(qn, "wq" if qn == "q" else "wk", npr)
        emit_recip(pipe[-1])
        norm_finish(pipe.pop(0))
        norm_finish(pipe.pop(0))
        emit_proj(NQC - 1, 0, 8)
    return nc


_NC = None


def _get_nc():
    global _NC
    if _NC is None:
        _patch_walrus_wait_limit()
        _NC = build_kernel()
    return _NC


def _host_tri():
    t = np.triu(np.full((128, 128), -30000.0, dtype=np.float32), 1)
    i = np.eye(128, dtype=np.float32)
    return np.concatenate([t, i], axis=1).astype(ml_dtypes.bfloat16)


def kernel(x, w_qkv, w_out, _trace=False, _trace_kwargs=None):
    x = np.asarray(x, dtype=np.float32)
    w_qkv = np.asarray(w_qkv, dtype=np.float32)
    w_out = np.asarray(w_out, dtype=np.float32)
    nc = _get_nc()

    tri = _host_tri()
    in_maps = []
    for c in range(NCORES):
        b, g = c // 2, c % 2
        cols = slice(g * FPC, (g + 1) * FPC)
        wq_ = w_qkv[:, 0 * DIM:1 * DIM][:, cols]
        wk_ = w_qkv[:, 1 * DIM:2 * DIM][:, cols]
        wv_ = w_qkv[:, 2 * DIM:3 * DIM][:, cols]
        in_maps.append({
            "xT": np.ascontiguousarray(x[b].T).astype(ml_dtypes.bfloat16),
            "wqkv": np.concatenate([wq_, wk_, wv_], axis=1)
                      .astype(ml_dtypes.bfloat16),
            "wo": w_out[g * FPC:(g + 1) * FPC, :].astype(ml_dtypes.bfloat16),
            "tri": tri,
        })

    res = run_bass_kernel_spmd(
        nc, in_maps, core_ids=list(range(NCORES)),
        trace=_trace, **(_trace_kwargs or {}))
    out = np.empty((4, SEQ, DIM), dtype=np.float32)
    for b in range(4):
        out[b] = (res.results[2 * b]["outT"].astype(np.float32)
                  + res.results[2 * b + 1]["outT"].astype(np.float32)).T
    if _trace:
        kernel.last_results = res
    return out


# revision 28
# speedup vs baseline: 1.0251x; 1.0073x over previous
"""Causal MHA (batch=4, seq=2048, dim=1024, 16 heads x 64) on 8 TRN2 NeuronCores.

Sharding: core c handles batch b = c//2 and head-group g = c%2 (8 heads).
Each core computes QKV projections for its heads, causal attention, and a
partial output projection over its 512 features. The host sums the two
partial projections per batch and transposes back.

All matmuls run in bf16 (fp32 PSUM accumulate); softmax runs without max
subtraction (logits are bounded ~|8|), with the row sums produced by an
extra ones-column appended to V during the PV matmul.

Schedule: per (head-pair, q-chunk) the softmax-normalization chain
(ln/exp recip of the PV sums row, a rank-2 selector matmul that
broadcasts both halves' recip rows across 128 partitions, and the DVE
normalize-multiplies) is pipelined one pair behind, emitted between the
next pair's off-diagonal sims and its diagonal sims, so neither PE nor
the DVE mask-mul FIFO ever waits on it. Diagonal tiles stream only
their valid q columns through the PE on both the sim and PV matmuls.
"""
import sys

sys.path.insert(0, "/opt/trn_rl_repo")

import json
import numpy as np
import ml_dtypes
from contextlib import ExitStack

import concourse.bass as bass
import concourse.tile as tile
from concourse import mybir
from concourse import bass_utils as _bu
from concourse.bass_utils import run_bass_kernel_spmd

LDW_OPT = False  # walrus ldw-opt rejects bass-emitted Ldweights outright

BF16 = mybir.dt.bfloat16
F32 = mybir.dt.float32
F32R = mybir.dt.float32r
Exp = mybir.ActivationFunctionType.Exp
Ln = mybir.ActivationFunctionType.Ln

DIM = 1024
SEQ = 2048
NH = 16          # total heads
HPC = 8          # heads per core
DH = 64          # head dim
SCALE = DH ** -0.5
NCORES = 8
FPC = HPC * DH   # features per core = 512
NKT = SEQ // 128   # 16 k-tiles of 128
NQC = SEQ // 512   # 4 q-chunks of 512
VSTRIDE = DH + 2   # 66: V columns per head incl. ones col + pad

_WALRUS_PATCHED = False


def _patch_walrus_wait_limit():
    """This container's walrus rejects >1 sem wait per instruction
    (CoreV3 setupSyncWait). Tile's tail drain carries one wait per live
    proc; split the extras into preceding single-wait Drain carriers at
    BIR-JSON serialization time."""
    global _WALRUS_PATCHED
    if _WALRUS_PATCHED:
        return
    _WALRUS_PATCHED = True

    if LDW_OPT:
        orig_run = _bu.run_command

        def run_patched(cmd, *a, **k):
            cmd = ["--enable-ldw-opt=true" if c == "--enable-ldw-opt=false" else c
                   for c in cmd]
            return orig_run(cmd, *a, **k)

        _bu.run_command = run_patched

    orig = bass.Bass.to_json_bytes

    def _merge_ldw_halves(insts):
        """Fold row-tiled Ldweights pairs ([64,128] at row 0 + [64,128] at
        row 64 of the same tensor) into one [128,128] load carrying both
        halves' waits."""
        out = []
        pend = None  # (index_in_out, inst) of a candidate row-0 half
        for inst in insts:
            op = inst["opcode"]
            if inst.get("engine") != "PE":
                out.append(inst)
                continue
            if op == "Ldweights" and inst.get("tile_size") == [64, 128]:
                ap = inst["ins"][0].get("ap")
                if inst.get("tile_position") == [0, 0] and ap and ap[0][1] == 64:
                    out.append(inst)
                    pend = (len(out) - 1, inst)
                    continue
                if (pend is not None
                        and inst.get("tile_position") == [64, 0] and ap
                        and ap[0][1] == 64):
                    a = pend[1]
                    aap = a["ins"][0]["ap"]
                    same = (a["ins"][0].get("memref") == inst["ins"][0].get("memref")
                            and aap[0][0] == ap[0][0] and aap[1] == ap[1]
                            and inst["ins"][0].get("offset", 0)
                            == a["ins"][0].get("offset", 0) + 64 * aap[0][0])
                    b_si = inst.get("sync_info") or {}
                    if same and not b_si.get("on_update"):
                        aap[0][1] = 128
                        a["tile_size"] = [128, 128]
                        a.setdefault("sync_info", {"on_update": [], "on_wait": []})
                        a["sync_info"].setdefault("on_wait", [])
                        a["sync_info"]["on_wait"].extend(b_si.get("on_wait") or [])
                        pend = None
                        continue
                out.append(inst)
                pend = None
            else:
                if op not in ("Matmult", "NoOp"):
                    pend = None
                out.append(inst)
        return out

    def patched(self, *a, **k):
        d = json.loads(orig(self, *a, **k))
        for f in d["functions"]:
            for bb in f["blocks"]:
                bb["instructions"] = _merge_ldw_halves(bb["instructions"])
                out = []
                last_ldw = None  # (key, still_valid)
                for inst in bb["instructions"]:
                    si = inst.get("sync_info")
                    ow = (si or {}).get("on_wait") or []
                    op = inst["opcode"]

                    def emit_carriers(waits):
                        for j, w in enumerate(waits):
                            out.append({
                                "name": f"{inst['name']}__w{j}",
                                "opcode": "NoOp",
                                "engine": inst["engine"],
                                "ins": [], "outs": [],
                                "debug": inst.get("debug", 0),
                                "sync_info": {"on_update": [], "on_wait": [w]},
                            })

                    # drop a Ldweights identical to the previous one when only
                    # Matmult/NoOp sit between (weights already resident);
                    # also fold the row-tiled [64,128]+[64,128] half-pair into
                    # the single [128,128] load emitted by _merge_ldw_halves
                    if op == "Ldweights" and inst["engine"] == "PE":
                        key = json.dumps(
                            [inst.get("ins"), inst.get("tile_position"),
                             inst.get("tile_size")], sort_keys=True)
                        if last_ldw == key and not (si or {}).get("on_update"):
                            emit_carriers(ow)
                            continue
                        last_ldw = key
                    elif inst["engine"] == "PE" and op not in ("Matmult", "NoOp"):
                        last_ldw = None

                    if len(ow) > 1:
                        emit_carriers(ow[:-1])
                        si["on_wait"] = [ow[-1]]
                    out.append(inst)
                bb["instructions"] = out
        return json.dumps(d).encode()

    bass.Bass.to_json_bytes = patched


def build_kernel():
    nc = bass.Bass()
    xT = nc.declare_dram_parameter("xT", [DIM, SEQ], BF16, isOutput=False)
    wqkv = nc.declare_dram_parameter("wqkv", [DIM, 3 * FPC], BF16,
                                     isOutput=False)
    wo = nc.declare_dram_parameter("wo", [FPC, DIM], BF16, isOutput=False)
    # tri[:, 0:128] = strict upper-triangular -30000 (causal mask addend,
    # lhsT orientation); tri[:, 128:256] = identity (its moving operand)
    tri = nc.declare_dram_parameter("tri", [128, 256], BF16, isOutput=False)
    outT = nc.declare_dram_parameter("outT", [DIM, SEQ], BF16, isOutput=True)

    with tile.TileContext(nc) as tc, ExitStack() as ctx:
        persist = ctx.enter_context(tc.tile_pool(name="persist", bufs=1))
        work = ctx.enter_context(tc.tile_pool(name="work", bufs=4))
        pt_pool = ctx.enter_context(tc.tile_pool(name="pt", bufs=1))
        ps_mm = ctx.enter_context(tc.tile_pool(name="ps_mm", bufs=2, space="PSUM"))
        ps_s = ctx.enter_context(tc.tile_pool(name="ps_s", bufs=2, space="PSUM"))
        ps_o = ctx.enter_context(tc.tile_pool(name="ps_o", bufs=2, space="PSUM"))

        # ---- load inputs. One batched DMA per tensor (xT in two halves):
        # a [di*128+p, c] HBM row block lands at SBUF [p, di*W + c], so a
        # per-di "tile" is a column slice of one wide tile. Few DMA issues
        # (the HWDGE ring serializes issue at ~0.6us each) and the first
        # QKV chain starts as soon as wq + the first xT half land. -------
        w_wide = persist.tile([128, 24 * FPC], BF16, tag="wqkv", name="wqkv")
        xT_wide = persist.tile([128, 8 * SEQ], BF16, tag="xT", name="xT")
        wo_wide = persist.tile([128, 4 * DIM], BF16, tag="wo", name="wo")
        tri_sb = persist.tile([128, 256], BF16, tag="tri", name="tri")
        _woff = {"wq": 0, "wk": FPC, "wv": 2 * FPC}

        def wsl(name, di):       # [128,FPC] view of weight block di
            o = di * 3 * FPC + _woff[name]
            return w_wide[:, o:o + FPC]

        def xsl(di):             # [128,SEQ] view of xT block di
            return xT_wide[:, di * SEQ:(di + 1) * SEQ]

        # weights on the scalar HWDGE ring, xT (in 4 chunks so the QKV
        # chains can start on the first k-quarter) on the sync ring —
        # a big DMA's transfer occupies its issuing queue, so two rings
        # let weights and activations stream in parallel.
        # xT in 4 chunks on the sync HWDGE ring; weights as per-di tiles
        # (contiguous HBM rows -> full DMA bandwidth, unlike the strided
        # batched form) split across the gpsimd SWDGE and scalar HWDGE
        # queues so issue serialization doesn't delay late tensors.
        for qt in range(4):
            eng = nc.sync if qt < 2 else nc.scalar
            eng.dma_start(
                xT_wide[:].rearrange("p (di c) -> p di c", di=8)[:, 2 * qt:2 * qt + 2],
                xT.ap().rearrange("(di p) c -> p di c", di=8)[:, 2 * qt:2 * qt + 2])
        for di in range(8):
            nc.gpsimd.dma_start(
                w_wide[:, di * 3 * FPC:(di + 1) * 3 * FPC],
                wqkv.ap()[di * 128:(di + 1) * 128, :])
        nc.scalar.dma_start(
            wo_wide[:].rearrange("p (fi c) -> p fi c", fi=4),
            wo.ap().rearrange("(fi p) c -> p fi c", fi=4))
        nc.scalar.dma_start(tri_sb[:], tri.ap())
        wo_sb = [wo_wide[:, fi * DIM:(fi + 1) * DIM] for fi in range(4)]
        ones64 = persist.tile([1, DH], BF16, tag="ones64")
        nc.gpsimd.memset(ones64[:], 1.0)

        # ---- stage B: QKV projections -----------------------------------
        qk_sb = {"q": [], "k": []}
        for qn in ("q", "k"):
            for fi in range(4):
                qk_sb[qn].append(
                    persist.tile([128, SEQ], BF16, tag=f"{qn}{fi}",
                                 name=f"{qn}{fi}"))
        v_sb = [persist.tile([128, HPC * VSTRIDE], BF16, tag=f"v{ti}",
                             name=f"v{ti}") for ti in range(NKT)]

        def chains4():
            # 4 simultaneous [128,512] accumulators: 2 from the mm tag plus 2
            # borrowed from the (momentarily idle) o tag — keeps weights
            # stationary across 4 matmuls so the LDW dedupe can drop 3 of 4
            return [ps_mm.tile([128, 512], F32, tag="mm", name="ch0"),
                    ps_mm.tile([128, 512], F32, tag="mm", name="ch1"),
                    ps_o.tile([128, 512], F32, tag="o", name="ch2"),
                    ps_o.tile([128, 512], F32, tag="o", name="ch3")]

        def emit_qk(qn, wn, fi):
            # Q, K in [feature, token] layout (w stationary, xT moving)
            t = qk_sb[qn][fi]
            ch = chains4()
            for di in range(8):
                for tck in range(4):
                    nc.tensor.matmul(
                        ch[tck][:], wsl(wn, di)[:, fi * 128:(fi + 1) * 128],
                        xsl(di)[:, tck * 512:(tck + 1) * 512],
                        start=(di == 0), stop=(di == 7))
            for tck in range(4):
                nc.vector.tensor_copy(t[:, tck * 512:(tck + 1) * 512], ch[tck][:])

        def emit_v(ti):
            # V in [token, feature] layout (xT stationary, wv moving), strided
            # into VSTRIDE-blocks with a ones column per head
            t = v_sb[ti]
            p = ps_mm.tile([128, 512], F32, tag="mm", name="p_v")
            for di in range(8):
                nc.tensor.matmul(
                    p[:], xsl(di)[:, ti * 128:(ti + 1) * 128],
                    wsl("wv", di),
                    start=(di == 0), stop=(di == 7))
            dst = t[:].rearrange("p (h c) -> p h c", h=HPC)[:, :, 0:DH]
            src = p[:].rearrange("p (h c) -> p h c", h=HPC)
            nc.vector.tensor_copy(dst, src)
            nc.gpsimd.memset(
                t[:].rearrange("p (h c) -> p h c", h=HPC)[:, :, DH:DH + 1], 1.0)

        ot_sb = [persist.tile([128, SEQ], BF16, tag=f"ot{fi}", name=f"ot{fi}")
                 for fi in range(4)]
        pts_map = {}

        def emit_sim(pr, ci, j0, j1):
            # S^T strips + exp into pt tiles for (head pair pr, q-chunk ci),
            # k-tiles j0..j1-1. Diagonal tiles (r >= 1) stream only their
            # valid q columns.
            q0 = ci * 512
            pts = pts_map.setdefault((pr, ci), [])
            for j in range(j0, j1):
                r = j - 4 * ci
                c0 = 128 * r if r > 0 else 0   # first valid q col in chunk
                ps = ps_s.tile([128, 1024], F32, tag="s", name="ps_st")
                for half in range(2):   # head A / head B, row-tiled
                    nc.tensor.matmul(
                        ps[:, half * 512 + c0:(half + 1) * 512],
                        qk_sb["k"][pr][half * 64:(half + 1) * 64,
                                       j * 128:(j + 1) * 128],
                        qk_sb["q"][pr][half * 64:(half + 1) * 64,
                                       q0 + c0:q0 + 512],
                        start=True, stop=True)
                pt = pt_pool.tile([128, 1024], BF16, tag=f"pt{j}", name="pt",
                                  bufs=2 if j < 14 else 1)
                pts.append(pt)
                if r < 0:
                    nc.scalar.activation(pt[:], ps[:], Exp, scale=SCALE)
                else:
                    # diagonal tile: add -30000 above the in-block diagonal
                    # (exp then underflows to exactly 0), zero the columns
                    # left of the valid range (PV streams the full chunk on
                    # its closing matmul)
                    for half in range(2):
                        nc.tensor.matmul(
                            ps[:, half * 512 + c0:half * 512 + c0 + 128],
                            tri_sb[:, 0:128], tri_sb[:, 128:256],
                            start=False, stop=True)
                    pt3 = pt[:].rearrange("p (b w) -> p b w", b=2)[:, :, c0:]
                    ps3 = ps[:].rearrange("p (b w) -> p b w", b=2)[:, :, c0:]
                    if r > 0:
                        nc.gpsimd.memset(
                            pt[:].rearrange("p (b w) -> p b w", b=2)[:, :, 0:c0],
                            0.0)
                    nc.scalar.activation(pt3, ps3, Exp, scale=SCALE)

        def emit_pv(pr, ci):
            # PV: V_aug stationary [128k, 65], P^T moving.
            # Output O^T_aug [65, 512q]: rows 0:64 = O^T, row 64 = sums.
            # Diagonal tiles r in {1,2} stream only valid columns; the last
            # tile streams full width (its masked cols are zero in pt) so
            # every PSUM element's accumulation closes with stop=True.
            # The UNNORMALIZED O^T is cast straight into ot_sb (freeing the
            # po bank as soon as the recip-input ln also reads it);
            # normalization happens in place two pairs later.
            q0 = ci * 512
            njs = 4 * ci + 4
            pts = pts_map.pop((pr, ci))
            pos = []
            for half in range(2):
                h = 2 * pr + half
                fi, row = h // 2, (h % 2) * 64
                po = ps_o.tile([DH + 1, 512], F32, tag="o", name="po")
                pos.append(po)
                for j in range(njs):
                    r = j - 4 * ci
                    c0 = 128 * r if (1 <= r and j < njs - 1) else 0
                    nc.tensor.matmul(
                        po[:, c0:],
                        v_sb[j][:, h * VSTRIDE:h * VSTRIDE + DH + 1],
                        pts[j][:, half * 512 + c0:(half + 1) * 512],
                        start=(j == 0), stop=(j == njs - 1))
                nc.vector.tensor_copy(
                    ot_sb[fi][row:row + 64, q0:q0 + 512], po[0:DH, :])
            return [pr, ci, pos, None]

        def emit_recip(rec):
            # ln then exp(-x) of both sums rows (same ACT table set as the
            # softmax exps). Emitted inside the NEXT pair's exp stream so
            # the ACT never stalls waiting for the PV to finish.
            pr, ci, pos, _ = rec
            lrow = work.tile([1, 1024], F32, tag="lrow", name="lrow", bufs=2)
            rrow = work.tile([1, 1024], BF16, tag="rrow", name="rrow", bufs=2)
            for half in range(2):
                nc.scalar.activation(lrow[0:1, half * 512:(half + 1) * 512],
                                     pos[half][DH:DH + 1, :], Ln)
            nc.scalar.activation(rrow[:], lrow[:], Exp, scale=-1.0)
            rec[2] = None
            rec[3] = rrow

        def norm_finish(rec):
            # rank-1 matmuls broadcast each half's recip row across 64
            # partitions, then DVE scales O^T in place. Runs two pairs
            # after the PV, so the recip rows are always ready.
            pr, ci, _, rrow = rec
            q0 = ci * 512
            for half in range(2):
                h = 2 * pr + half
                fi, row = h // 2, (h % 2) * 64
                rb_ps = ps_mm.tile([DH, 512], F32, tag="mm", name="rb_ps")
                nc.tensor.matmul(
                    rb_ps[:], ones64[:],
                    rrow[0:1, half * 512:(half + 1) * 512],
                    start=True, stop=True)
                ot = ot_sb[fi][row:row + 64, q0:q0 + 512]
                nc.vector.tensor_mul(ot, ot, rb_ps[:])

        def emit_proj(ci, e0, e1):
            # projection for chunk ci's columns (all pairs' OT normalized)
            for ei in range(e0, e1):
                p = ps_mm.tile([128, 512], F32, tag="mm", name="p_proj")
                for fi in range(4):
                    nc.tensor.matmul(
                        p[:], wo_sb[fi][:, ei * 128:(ei + 1) * 128],
                        ot_sb[fi][:, ci * 512:(ci + 1) * 512],
                        start=(fi == 0), stop=(fi == 3))
                os_ = work.tile([128, 512], BF16, tag="os", name="os", bufs=2)
                nc.vector.tensor_copy(os_[:], p[:])
                eng = nc.sync if ei % 2 == 0 else nc.scalar
                eng.dma_start(
                    outT.ap()[ei * 128:(ei + 1) * 128,
                              ci * 512:(ci + 1) * 512], os_[:])

        # Two-phase woven schedule balancing PE-heavy projection work
        # against the ACT-bound exp stream. Phase 1: per head-pair pr, its
        # chunks 0..2, with the NEXT pair's Q/K chains woven in (they only
        # use the mm psum tag, so they slot into exp-paced PE bubbles and
        # the next block's sims start without a projection stall); phase
        # 2: the four chunk-3 pairs, PE-filled with V group 3 and the
        # deferred output projections. Within a pair: off-diagonal sims,
        # previous pair's recip (ACT), V fills, pair n-2's norm_finish,
        # projection fill, Q/K weave, diagonal sims, PV + casts.
        order = [(pr, ci) for pr in range(4) for ci in range(3)]
        order += [(pr, 3) for pr in range(4)]
        v_fill = {(0, 0): [0, 1, 2, 3], (0, 1): [4, 5, 6, 7],
                  (0, 2): [8, 9, 10, 11], (0, 3): [12, 13, 14, 15]}
        proj_fill = {(1, 3): 0, (2, 3): 1, (3, 3): 2}
        # next block's Q/K chains, emitted right after this pair's PV so
        # the next block's sims never wait on them; chains4 borrows ps_o,
        # so the pending recip (reading the last po) must flush first.
        qk_post = {(0, 0): ("q", 1), (0, 1): ("k", 1),
                   (1, 0): ("q", 2), (1, 1): ("k", 2),
                   (2, 0): ("q", 3), (2, 1): ("k", 3)}
        emit_qk("q", "wq", 0)
        emit_qk("k", "wk", 0)
        pipe = []   # records awaiting recip (last) / norm_finish (first)
        for pr, ci in order:
            emit_sim(pr, ci, 0, 4 * ci)
            if pipe and pipe[-1][3] is None:
                emit_recip(pipe[-1])
            if len(pipe) >= 2:
                norm_finish(pipe.pop(0))
            if (pr, ci) in proj_fill:
                emit_proj(proj_fill[pr, ci], 0, 8)
            emit_sim(pr, ci, 4 * ci, 4 * ci + 4)
            for ti in v_fill.get((pr, ci), []):
                emit_v(ti)
            pipe.append(emit_pv(pr, ci))
            if (pr, ci) in qk_post:
                emit_recip(pipe[-1])
                qn, npr = qk_post[pr, ci]
                emit_qk(qn, "wq" if qn == "q" else "wk", npr)
        emit_recip(pipe[-1])
        norm_finish(pipe.pop(0))
        norm_finish(pipe.pop(0))
        emit_proj(NQC - 1, 0, 8)
    return nc


_NC = None


def _get_nc():
    global _NC
    if _NC is None:
        _patch_walrus_wait_limit()
        _NC = build_kernel()
    return _NC


def _host_tri():
    t = np.triu(np.full((128, 128), -30000.0, dtype=np.float32), 1)
    i = np.eye(128, dtype=np.float32)
    return np.concatenate([t, i], axis=1).astype(ml_dtypes.bfloat16)


def kernel(x, w_qkv, w_out, _trace=False, _trace_kwargs=None):
    x = np.asarray(x, dtype=np.float32)
    w_qkv = np.asarray(w_qkv, dtype=np.float32)
    w_out = np.asarray(w_out, dtype=np.float32)
    nc = _get_nc()

    tri = _host_tri()
    in_maps = []
    for c in range(NCORES):
        b, g = c // 2, c % 2
        cols = slice(g * FPC, (g + 1) * FPC)
        wq_ = w_qkv[:, 0 * DIM:1 * DIM][:, cols]
        wk_ = w_qkv[:, 1 * DIM:2 * DIM][:, cols]
        wv_ = w_qkv[:, 2 * DIM:3 * DIM][:, cols]
        in_maps.append({
            "xT": np.ascontiguousarray(x[b].T).astype(ml_dtypes.bfloat16),
            "wqkv": np.concatenate([wq_, wk_, wv_], axis=1)
                      .astype(ml_dtypes.bfloat16),
            "wo": w_out[g * FPC:(g + 1) * FPC, :].astype(ml_dtypes.bfloat16),
            "tri": tri,
        })

    res = run_bass_kernel_spmd(
        nc, in_maps, core_ids=list(range(NCORES)),
        trace=_trace, **(_trace_kwargs or {}))
    out = np.empty((4, SEQ, DIM), dtype=np.float32)
    for b in range(4):
        out[b] = (res.results[2 * b]["outT"].astype(np.float32)
                  + res.results[2 * b + 1]["outT"].astype(np.float32)).T
    if _trace:
        kernel.last_results = res
    return out
